# revision 19
# baseline (speedup 1.0000x reference)
"""CARAFE-Downsample Trainium2 kernel (8 NeuronCores, data-parallel over batch).

Problem (hardcoded shapes): x [8, 256, 128, 128] f32; 1x1-conv compressor ->
cx [8, 64, 128, 128]; 3x3 stride-2 conv encoder -> mask [8, 25, 64, 64];
softmax(mask * exp(p)) over the 25 taps; 5x5 stride-2 weighted reassembly of x
-> out [8, 256, 64, 64].

Strategy (v1.1):
 - one sample per core (B == n_cores == 8).
 - Pixel-block layout: output block k (k in 0..31) holds the 128 output pixels
   {(h', w') : h' in {k, k+32}, w' in 0..63} on the 128 SBUF partitions
   (p = half*64 + w').  Every 5x5 tap is a single full-width PE matmul with a
   diagonal stationary matrix diag(e_t) against a host-pregathered slab.
 - diag matrices for all 25 taps of a block are written by ONE DVE
   copy_predicated into a [128, 25, 128] tile whose zero background is
   memset once per rotation buffer and never dirtied (the predicated write
   touches only the diagonal cells, which repeat every rotation).
 - softmax normalization is folded into the final psum->SBUF copy
   (ACT Copy with per-partition scale 1/sum(exp)); diag values are raw exps.
 - mask path (compressor + encoder) runs in bf16 on the TensorEngine;
   exp(power_p) folded into encoder weights on host.
 - DMA: xc loads batched in pairs, outputs batched 4 blocks per store,
   first slab tiles prefetched during phase A.
"""

import numpy as np
import ml_dtypes

import concourse.bass as bass
import concourse.bacc as bacc
import concourse.tile as tile
from concourse import mybir
from concourse.bass_utils import run_bass_kernel_spmd

# -- problem constants (hardcoded per spec) ---------------------------------
B, C, H, W = 8, 256, 128, 128
CC = 64           # compressed channels
KK = 5            # CARAFE window
HP = WP = 64      # output spatial
NB = 32           # pixel blocks per sample
NCORES = 8

X_DTYPE = "bf16"
MASK_DTYPE = "bf16"

_DTM = {"f32": mybir.dt.float32, "bf16": mybir.dt.bfloat16}
_NPM = {"f32": np.float32, "bf16": ml_dtypes.bfloat16}
DTX, DTK = _DTM[X_DTYPE], _DTM[MASK_DTYPE]
NPX, NPK = _NPM[X_DTYPE], _NPM[MASK_DTYPE]
F32 = mybir.dt.float32

# tap -> (slab index, block-row offset). slab sl = oh*5 + j holds x rows of
# parity oh, cols (j-2)+2*w'' (zero padded), block rows kk = -1..32.
def _tap_table():
    taps = []
    for i in range(KK):
        oh = (i - 2) % 2
        dh = (i - 2 - oh) // 2
        for j in range(KK):
            taps.append((i * 5 + j, oh * 5 + j, dh))
    return taps

_TAPS = _tap_table()

NSLAB_EARLY = 9   # slab tiles prefetched during phase A (== slab pool bufs)


def _build_nc():
    nc = bacc.Bacc(None, target_bir_lowering=False, debug=False)

    xc_d = nc.declare_dram_parameter("xc", [128, 2, H * W], DTK, isOutput=False)
    sl_d = nc.declare_dram_parameter("slabs", [34, 128, 10, C], DTX, isOutput=False)
    wc_d = nc.declare_dram_parameter("wc", [2, 128, CC], DTK, isOutput=False)
    bc_d = nc.declare_dram_parameter("bc", [CC, 1], F32, isOutput=False)
    wt_d = nc.declare_dram_parameter("wt", [CC, 9, 25], DTK, isOutput=False)
    be_d = nc.declare_dram_parameter("be", [25, 1], F32, isOutput=False)
    id_d = nc.declare_dram_parameter("idn", [25, 25], DTK, isOutput=False)
    c0_d = nc.declare_dram_parameter("c0", [128, 128], mybir.dt.uint8,
                                     isOutput=False)
    out_d = nc.declare_dram_parameter("out", [NB, 128, C], F32, isOutput=True)

    CXW = 130  # padded cx row length; cx_pad[c, r*130 + col], r/col offset by 1

    with tile.TileContext(nc) as tc:
        with (
            tc.tile_pool(name="consts", bufs=1) as consts,
            tc.tile_pool(name="xcin", bufs=3) as xcin,
            tc.tile_pool(name="cx", bufs=1) as cxpool,
            tc.tile_pool(name="psA", bufs=2, space="PSUM") as psA,
            tc.tile_pool(name="psM", bufs=2, space="PSUM") as psM,
            tc.tile_pool(name="psT", bufs=1, space="PSUM") as psT,
            tc.tile_pool(name="psO", bufs=3, space="PSUM") as psO,
            tc.tile_pool(name="soft", bufs=6) as soft,
            tc.tile_pool(name="slab", bufs=NSLAB_EARLY) as slabp,
            tc.tile_pool(name="diag", bufs=4) as diagp,
            tc.tile_pool(name="outp", bufs=2) as outp,
        ):
            # ---- constants / weights ----
            wc_sb = consts.tile([128, 2, CC], DTK)
            nc.sync.dma_start(out=wc_sb, in_=wc_d[:, :, :].rearrange("c p m -> p c m"))
            wt_sb = consts.tile([CC, 9, 25], DTK)
            nc.sync.dma_start(out=wt_sb, in_=wt_d[:, :, :])
            bc_sb = consts.tile([CC, 1], F32)
            nc.sync.dma_start(out=bc_sb, in_=bc_d[:, :])
            be_sb = consts.tile([25, 1], F32)
            nc.sync.dma_start(out=be_sb, in_=be_d[:, :])
            id_sb = consts.tile([25, 25], DTK)
            nc.sync.dma_start(out=id_sb, in_=id_d[:, :])
            c0_sb = consts.tile([128, 128], mybir.dt.uint8)
            nc.sync.dma_start(out=c0_sb, in_=c0_d[:, :])

            # ---- cx_pad (compressor output, 1-px zero ring, flat layout) ----
            cx_pad = cxpool.tile([CC, CXW * CXW], DTK)
            cp = cx_pad[:, :]
            zrow = consts.tile([CC, CXW], DTK)
            nc.vector.memset(zrow, 0.0)
            nc.scalar.copy(out=cp[:, 0:CXW], in_=zrow[:, :])
            nc.scalar.copy(
                out=bass.AP(tensor=cp.tensor, offset=cp.offset + CXW,
                            ap=[cp.ap[0], [CXW, 129], [1, 1]]),
                in_=zrow[:, 0:129],
            )

            tc.strict_bb_all_engine_barrier()

            # ---- phase A: compressor 1x1 conv (PE, bf16), xc in pairs ----
            slab_tiles = []

            def load_slab():
                kk = len(slab_tiles)
                st = slabp.tile([128, 10, C], DTX, tag="sl")
                nc.sync.dma_start(out=st, in_=sl_d[kk, :, :, :])
                slab_tiles.append(st)

            for jb in range(16):
                xt = xcin.tile([128, 2, 1024], DTK)
                nc.sync.dma_start(
                    out=xt, in_=xc_d[:, :, jb * 1024:(jb + 1) * 1024])
                if 3 <= jb <= 11:   # prefetch first slab tiles behind xc
                    load_slab()
                for jj in range(2):
                    j = 2 * jb + jj
                    pm = psA.tile([CC, 512], F32)
                    nc.tensor.matmul(pm, lhsT=wc_sb[:, 0, :],
                                     rhs=xt[:, 0, jj * 512:(jj + 1) * 512],
                                     start=True, stop=False)
                    nc.tensor.matmul(pm, lhsT=wc_sb[:, 1, :],
                                     rhs=xt[:, 1, jj * 512:(jj + 1) * 512],
                                     start=False, stop=True)
                    dst = bass.AP(tensor=cp.tensor,
                                  offset=cp.offset + (4 * j + 1) * CXW + 1,
                                  ap=[cp.ap[0], [CXW, 4], [1, 128]])
                    nc.scalar.activation(
                        out=dst,
                        in_=pm[:, :].rearrange("p (r n) -> p r n", n=128),
                        func=mybir.ActivationFunctionType.Identity,
                        bias=bc_sb[:, :])

            # ---- phase B: encoder 3x3/s2 conv -> m_all [25, 4096] (bf16) ----
            m_all = cxpool.tile([25, HP * WP], DTK)
            for j2 in range(8):
                pmM = psM.tile([25, 512], F32)
                ti = 0
                for di in range(3):
                    for dj in range(3):
                        rhs = bass.AP(
                            tensor=cp.tensor,
                            offset=cp.offset + (16 * j2 + di) * CXW + dj,
                            ap=[cp.ap[0], [2 * CXW, 8], [2, 64]],
                        )
                        nc.tensor.matmul(pmM, lhsT=wt_sb[:, ti, :], rhs=rhs,
                                         start=(ti == 0), stop=(ti == 8))
                        ti += 1
                nc.scalar.activation(out=m_all[:, j2 * 512:(j2 + 1) * 512],
                                     in_=pmM,
                                     func=mybir.ActivationFunctionType.Identity,
                                     bias=be_sb[:, :])

            # ---- phase C: per block: transpose + exp + 1/sum ----
            e_blocks, r_blocks = [], []
            for k in range(NB):
                e_k = soft.tile([128, 25], F32, tag="e")
                for half in range(2):
                    hcol = (k + 32 * half) * 64
                    pmT = psT.tile([64, 25], DTK)
                    nc.tensor.transpose(pmT, m_all[:, hcol:hcol + 64], id_sb[:, :])
                    nc.scalar.activation(out=e_k[half * 64:(half + 1) * 64, :],
                                         in_=pmT,
                                         func=mybir.ActivationFunctionType.Exp)
                r_k = soft.tile([128, 1], F32, tag="r")
                nc.vector.reduce_sum(out=r_k, in_=e_k, axis=mybir.AxisListType.X)
                nc.vector.reciprocal(out=r_k, in_=r_k)
                e_blocks.append(e_k)
                r_blocks.append(r_k)

            # ---- phase D: diag-matmul reassembly ----
            for _ in range(len(slab_tiles), 34):
                load_slab()

            tapmap = {t: (sl, dh) for (t, sl, dh) in _TAPS}
            c0_v = bass.AP(tensor=c0_sb.tensor, offset=c0_sb[:, :].offset,
                           ap=[c0_sb[:, :].ap[0], [0, 25], [1, 128]])

            # zero every physical diag buffer once; the predicated writes only
            # ever touch the diagonal cells, so the background stays zero
            for _ in range(4):
                Dz = diagp.tile([128, 25, 128], DTX, tag="diag")
                nc.vector.memset(Dz, 0.0)

            for k in range(NB):
                e_k, r_k = e_blocks[k], r_blocks[k]
                D_all = diagp.tile([128, 25, 128], DTX, tag="diag")
                ev = bass.AP(tensor=e_k.tensor, offset=e_k[:, :].offset,
                             ap=[e_k[:, :].ap[0], [1, 25], [0, 128]])
                nc.vector.copy_predicated(out=D_all, mask=c0_v, data=ev)
                po_t = psO.tile([128, C], F32)
                for t in range(25):
                    sl, dh = tapmap[t]
                    nc.tensor.matmul(po_t, lhsT=D_all[:, t, :],
                                     rhs=slab_tiles[k + dh + 1][:, sl, :],
                                     start=(t == 0), stop=(t == 24))
                if k % 4 == 0:
                    fin4 = outp.tile([128, 4, C], F32, tag="fin")
                nc.scalar.activation(out=fin4[:, k % 4, :], in_=po_t,
                                     func=mybir.ActivationFunctionType.Copy,
                                     scale=r_k[:, :])
                if k % 4 == 3:
                    nc.sync.dma_start(
                        out=out_d[k - 3:k + 1, :, :].rearrange("k p c -> p k c"),
                        in_=fin4)

    nc.compile()
    return nc


_NC_CACHE = None
LAST_RESULTS = None


def _get_nc():
    global _NC_CACHE
    if _NC_CACHE is None:
        _NC_CACHE = _build_nc()
    return _NC_CACHE


def _host_prep(x, w_comp, b_comp, w_enc, b_enc, power_p):
    """Build per-core input maps (numpy only)."""
    pe = float(np.exp(np.float64(power_p)))

    xc_all = np.ascontiguousarray(
        x.reshape(B, 2, 128, H * W).transpose(0, 2, 1, 3)).astype(NPK)

    # slabs [B, 34, 128, 10, C]
    xp = np.pad(x, ((0, 0), (0, 0), (2, 2), (2, 2)))  # [B, C, 132, 132]
    kk = np.arange(-1, 33)
    slabs = np.empty((B, 34, 128, 10, C), dtype=NPX)
    for oh in range(2):
        rows = (2 * kk[:, None] + 64 * np.arange(2)[None, :]) + oh + 2  # [34, 2]
        g0 = xp[:, :, rows, :]                     # [B, C, 34, 2, 132]
        for j in range(KK):
            g = g0[:, :, :, :, j:j + 128:2]        # [B, C, 34, 2, 64]
            slabs[:, :, :, oh * 5 + j, :] = (
                g.transpose(0, 2, 3, 4, 1).reshape(B, 34, 128, C))

    wc = np.ascontiguousarray(
        w_comp[:, :, 0, 0].T.reshape(2, 128, CC)).astype(NPK)
    bc = b_comp.reshape(CC, 1).astype(np.float32)
    wt = np.empty((CC, 9, 25), dtype=NPK)
    for di in range(3):
        for dj in range(3):
            wt[:, 3 * di + dj, :] = (pe * w_enc[:, :, di, dj]).T.astype(NPK)
    be = (pe * b_enc).reshape(25, 1).astype(np.float32)
    idn = np.eye(25, dtype=NPK)
    c0 = np.eye(128, dtype=np.uint8)

    in_maps = []
    for b in range(B):
        in_maps.append({
            "xc": np.ascontiguousarray(xc_all[b]),
            "slabs": np.ascontiguousarray(slabs[b]),
            "wc": wc, "bc": bc, "wt": wt, "be": be, "idn": idn, "c0": c0,
        })
    return in_maps


def kernel(x, w_comp, b_comp, w_enc, b_enc, power_p):
    x = np.asarray(x, dtype=np.float32)
    in_maps = _host_prep(np.asarray(x), np.asarray(w_comp), np.asarray(b_comp),
                         np.asarray(w_enc), np.asarray(b_enc),
                         np.asarray(power_p))
    nc = _get_nc()
    res = run_bass_kernel_spmd(nc, in_maps, list(range(NCORES)))
    global LAST_RESULTS
    LAST_RESULTS = res
    outs = np.stack([np.asarray(res.results[i]["out"]) for i in range(NCORES)])
    # [B, 32, 128, 256] -> [B, C, 64, 64]; h' = half*32 + k, p = half*64 + w'
    out = (outs.reshape(B, NB, 2, 64, C)
               .transpose(0, 4, 2, 1, 3)
               .reshape(B, C, HP, WP))
    return np.ascontiguousarray(out.astype(np.float32))


# revision 20
# speedup vs baseline: 1.0562x; 1.0562x over previous
"""CARAFE-Downsample Trainium2 kernel (8 NeuronCores, data-parallel over batch).

Problem (hardcoded shapes): x [8, 256, 128, 128] f32; 1x1-conv compressor ->
cx [8, 64, 128, 128]; 3x3 stride-2 conv encoder -> mask [8, 25, 64, 64];
softmax(mask * exp(p)) over the 25 taps; 5x5 stride-2 weighted reassembly of x
-> out [8, 256, 64, 64].

Strategy (v1.1):
 - one sample per core (B == n_cores == 8).
 - Pixel-block layout: output block k (k in 0..31) holds the 128 output pixels
   {(h', w') : h' in {k, k+32}, w' in 0..63} on the 128 SBUF partitions
   (p = half*64 + w').  Every 5x5 tap is a single full-width PE matmul with a
   diagonal stationary matrix diag(e_t) against a host-pregathered slab.
 - diag matrices for all 25 taps of a block are written by ONE DVE
   copy_predicated into a [128, 25, 128] tile whose zero background is
   memset once per rotation buffer and never dirtied (the predicated write
   touches only the diagonal cells, which repeat every rotation).
 - softmax normalization is folded into the final psum->SBUF copy
   (ACT Copy with per-partition scale 1/sum(exp)); diag values are raw exps.
 - mask path (compressor + encoder) runs in bf16 on the TensorEngine;
   exp(power_p) folded into encoder weights on host.
 - DMA: xc loads batched in pairs, outputs batched 4 blocks per store,
   first slab tiles prefetched during phase A.
"""

import numpy as np
import ml_dtypes

import concourse.bass as bass
import concourse.bacc as bacc
import concourse.tile as tile
from concourse import mybir
from concourse.bass_utils import run_bass_kernel_spmd

# -- problem constants (hardcoded per spec) ---------------------------------
B, C, H, W = 8, 256, 128, 128
CC = 64           # compressed channels
KK = 5            # CARAFE window
HP = WP = 64      # output spatial
NB = 32           # pixel blocks per sample
NCORES = 8

X_DTYPE = "bf16"
MASK_DTYPE = "bf16"

_DTM = {"f32": mybir.dt.float32, "bf16": mybir.dt.bfloat16}
_NPM = {"f32": np.float32, "bf16": ml_dtypes.bfloat16}
DTX, DTK = _DTM[X_DTYPE], _DTM[MASK_DTYPE]
NPX, NPK = _NPM[X_DTYPE], _NPM[MASK_DTYPE]
F32 = mybir.dt.float32

# tap -> (slab index, block-row offset). slab sl = oh*5 + j holds x rows of
# parity oh, cols (j-2)+2*w'' (zero padded), block rows kk = -1..32.
def _tap_table():
    taps = []
    for i in range(KK):
        oh = (i - 2) % 2
        dh = (i - 2 - oh) // 2
        for j in range(KK):
            taps.append((i * 5 + j, oh * 5 + j, dh))
    return taps

_TAPS = _tap_table()

NSLAB_EARLY = 9   # slab tiles prefetched during phase A (== slab pool bufs)


def _build_nc():
    nc = bacc.Bacc(None, target_bir_lowering=False, debug=False)

    xc_d = nc.declare_dram_parameter("xc", [128, 2, H * W], DTK, isOutput=False)
    sl_d = nc.declare_dram_parameter("slabs", [34, 128, 10, C], DTX, isOutput=False)
    wc_d = nc.declare_dram_parameter("wc", [2, 128, CC], DTK, isOutput=False)
    bc_d = nc.declare_dram_parameter("bc", [CC, 1], F32, isOutput=False)
    wt_d = nc.declare_dram_parameter("wt", [CC, 9, 25], DTK, isOutput=False)
    be_d = nc.declare_dram_parameter("be", [25, 1], F32, isOutput=False)
    id_d = nc.declare_dram_parameter("idn", [25, 25], DTK, isOutput=False)
    c0_d = nc.declare_dram_parameter("c0", [128, 128], mybir.dt.uint8,
                                     isOutput=False)
    out_d = nc.declare_dram_parameter("out", [NB, 128, C], F32, isOutput=True)

    CXW = 130  # padded cx row length; cx_pad[c, r*130 + col], r/col offset by 1

    with tile.TileContext(nc) as tc:
        with (
            tc.tile_pool(name="consts", bufs=1) as consts,
            tc.tile_pool(name="xcin", bufs=3) as xcin,
            tc.tile_pool(name="cx", bufs=1) as cxpool,
            tc.tile_pool(name="psA", bufs=2, space="PSUM") as psA,
            tc.tile_pool(name="psM", bufs=2, space="PSUM") as psM,
            tc.tile_pool(name="psT", bufs=1, space="PSUM") as psT,
            tc.tile_pool(name="psO", bufs=3, space="PSUM") as psO,
            tc.tile_pool(name="soft", bufs=6) as soft,
            tc.tile_pool(name="slab", bufs=NSLAB_EARLY) as slabp,
            tc.tile_pool(name="diag", bufs=4) as diagp,
            tc.tile_pool(name="outp", bufs=2) as outp,
        ):
            # ---- constants / weights ----
            wc_sb = consts.tile([128, 2, CC], DTK)
            nc.sync.dma_start(out=wc_sb, in_=wc_d[:, :, :].rearrange("c p m -> p c m"))
            wt_sb = consts.tile([CC, 9, 25], DTK)
            nc.sync.dma_start(out=wt_sb, in_=wt_d[:, :, :])
            bc_sb = consts.tile([CC, 1], F32)
            nc.sync.dma_start(out=bc_sb, in_=bc_d[:, :])
            be_sb = consts.tile([25, 1], F32)
            nc.sync.dma_start(out=be_sb, in_=be_d[:, :])
            id_sb = consts.tile([25, 25], DTK)
            nc.sync.dma_start(out=id_sb, in_=id_d[:, :])
            c0_sb = consts.tile([128, 128], mybir.dt.uint8)
            nc.sync.dma_start(out=c0_sb, in_=c0_d[:, :])

            # ---- cx_pad (compressor output, 1-px zero ring, flat layout) ----
            cx_pad = cxpool.tile([CC, CXW * CXW], DTK)
            cp = cx_pad[:, :]
            zrow = consts.tile([CC, CXW], DTK)
            nc.vector.memset(zrow, 0.0)
            nc.scalar.copy(out=cp[:, 0:CXW], in_=zrow[:, :])
            nc.scalar.copy(
                out=bass.AP(tensor=cp.tensor, offset=cp.offset + CXW,
                            ap=[cp.ap[0], [CXW, 129], [1, 1]]),
                in_=zrow[:, 0:129],
            )

            tc.strict_bb_all_engine_barrier()

            # ---- phase A: compressor 1x1 conv (PE, bf16), xc in pairs ----
            slab_tiles = []

            def load_slab():
                kk = len(slab_tiles)
                st = slabp.tile([128, 10, C], DTX, tag="sl")
                nc.sync.dma_start(out=st, in_=sl_d[kk, :, :, :])
                slab_tiles.append(st)

            for jb in range(16):
                xt = xcin.tile([128, 2, 1024], DTK)
                nc.sync.dma_start(
                    out=xt, in_=xc_d[:, :, jb * 1024:(jb + 1) * 1024])
                for jj in range(2):
                    j = 2 * jb + jj
                    pm = psA.tile([CC, 512], F32)
                    nc.tensor.matmul(pm, lhsT=wc_sb[:, 0, :],
                                     rhs=xt[:, 0, jj * 512:(jj + 1) * 512],
                                     start=True, stop=False)
                    nc.tensor.matmul(pm, lhsT=wc_sb[:, 1, :],
                                     rhs=xt[:, 1, jj * 512:(jj + 1) * 512],
                                     start=False, stop=True)
                    dst = bass.AP(tensor=cp.tensor,
                                  offset=cp.offset + (4 * j + 1) * CXW + 1,
                                  ap=[cp.ap[0], [CXW, 4], [1, 128]])
                    if j % 2 == 0:
                        nc.scalar.activation(
                            out=dst,
                            in_=pm[:, :].rearrange("p (r n) -> p r n", n=128),
                            func=mybir.ActivationFunctionType.Identity,
                            bias=bc_sb[:, :])
                    else:
                        nc.vector.tensor_scalar(
                            out=dst,
                            in0=pm[:, :].rearrange("p (r n) -> p r n", n=128),
                            scalar1=bc_sb[:, :], scalar2=None,
                            op0=mybir.AluOpType.add)

            # ---- phase B: encoder 3x3/s2 conv -> m_all [25, 4096] (bf16) ----
            m_all = cxpool.tile([25, HP * WP], DTK)
            for j2 in (0, 4, 1, 5, 2, 6, 3, 7):
                pmM = psM.tile([25, 512], F32)
                ti = 0
                for di in range(3):
                    for dj in range(3):
                        rhs = bass.AP(
                            tensor=cp.tensor,
                            offset=cp.offset + (16 * j2 + di) * CXW + dj,
                            ap=[cp.ap[0], [2 * CXW, 8], [2, 64]],
                        )
                        nc.tensor.matmul(pmM, lhsT=wt_sb[:, ti, :], rhs=rhs,
                                         start=(ti == 0), stop=(ti == 8))
                        ti += 1
                nc.scalar.activation(out=m_all[:, j2 * 512:(j2 + 1) * 512],
                                     in_=pmM,
                                     func=mybir.ActivationFunctionType.Identity,
                                     bias=be_sb[:, :])

            for _ in range(34):
                load_slab()

            # ---- phase C: per block: transpose + exp + 1/sum ----
            e_blocks, r_blocks = [], []
            for k in range(NB):
                e_k = soft.tile([128, 25], F32, tag="e")
                for half in range(2):
                    hcol = (k + 32 * half) * 64
                    pmT = psT.tile([64, 25], DTK)
                    nc.tensor.transpose(pmT, m_all[:, hcol:hcol + 64], id_sb[:, :])
                    nc.scalar.activation(out=e_k[half * 64:(half + 1) * 64, :],
                                         in_=pmT,
                                         func=mybir.ActivationFunctionType.Exp)
                r_k = soft.tile([128, 1], F32, tag="r")
                nc.vector.reduce_sum(out=r_k, in_=e_k, axis=mybir.AxisListType.X)
                nc.vector.reciprocal(out=r_k, in_=r_k)
                e_blocks.append(e_k)
                r_blocks.append(r_k)

            # ---- phase D: diag-matmul reassembly ----
            tapmap = {t: (sl, dh) for (t, sl, dh) in _TAPS}
            c0_v = bass.AP(tensor=c0_sb.tensor, offset=c0_sb[:, :].offset,
                           ap=[c0_sb[:, :].ap[0], [0, 25], [1, 128]])

            # zero every physical diag buffer once; the predicated writes only
            # ever touch the diagonal cells, so the background stays zero
            for _ in range(4):
                Dz = diagp.tile([128, 25, 128], DTX, tag="diag")
                nc.vector.memset(Dz, 0.0)

            for k in range(NB):
                e_k, r_k = e_blocks[k], r_blocks[k]
                D_all = diagp.tile([128, 25, 128], DTX, tag="diag")
                ev = bass.AP(tensor=e_k.tensor, offset=e_k[:, :].offset,
                             ap=[e_k[:, :].ap[0], [1, 25], [0, 128]])
                nc.vector.copy_predicated(out=D_all, mask=c0_v, data=ev)
                po_t = psO.tile([128, C], F32)
                for t in range(25):
                    sl, dh = tapmap[t]
                    nc.tensor.matmul(po_t, lhsT=D_all[:, t, :],
                                     rhs=slab_tiles[k + dh + 1][:, sl, :],
                                     start=(t == 0), stop=(t == 24))
                if k % 4 == 0:
                    fin4 = outp.tile([128, 4, C], F32, tag="fin")
                nc.scalar.activation(out=fin4[:, k % 4, :], in_=po_t,
                                     func=mybir.ActivationFunctionType.Copy,
                                     scale=r_k[:, :])
                if k % 4 == 3:
                    nc.sync.dma_start(
                        out=out_d[k - 3:k + 1, :, :].rearrange("k p c -> p k c"),
                        in_=fin4)

    nc.compile()
    return nc


_NC_CACHE = None
LAST_RESULTS = None


def _get_nc():
    global _NC_CACHE
    if _NC_CACHE is None:
        _NC_CACHE = _build_nc()
    return _NC_CACHE


def _host_prep(x, w_comp, b_comp, w_enc, b_enc, power_p):
    """Build per-core input maps (numpy only)."""
    pe = float(np.exp(np.float64(power_p)))

    xc_all = np.ascontiguousarray(
        x.reshape(B, 2, 128, H * W).transpose(0, 2, 1, 3)).astype(NPK)

    # slabs [B, 34, 128, 10, C]
    xp = np.pad(x, ((0, 0), (0, 0), (2, 2), (2, 2)))  # [B, C, 132, 132]
    kk = np.arange(-1, 33)
    slabs = np.empty((B, 34, 128, 10, C), dtype=NPX)
    for oh in range(2):
        rows = (2 * kk[:, None] + 64 * np.arange(2)[None, :]) + oh + 2  # [34, 2]
        g0 = xp[:, :, rows, :]                     # [B, C, 34, 2, 132]
        for j in range(KK):
            g = g0[:, :, :, :, j:j + 128:2]        # [B, C, 34, 2, 64]
            slabs[:, :, :, oh * 5 + j, :] = (
                g.transpose(0, 2, 3, 4, 1).reshape(B, 34, 128, C))

    wc = np.ascontiguousarray(
        w_comp[:, :, 0, 0].T.reshape(2, 128, CC)).astype(NPK)
    bc = b_comp.reshape(CC, 1).astype(np.float32)
    wt = np.empty((CC, 9, 25), dtype=NPK)
    for di in range(3):
        for dj in range(3):
            wt[:, 3 * di + dj, :] = (pe * w_enc[:, :, di, dj]).T.astype(NPK)
    be = (pe * b_enc).reshape(25, 1).astype(np.float32)
    idn = np.eye(25, dtype=NPK)
    c0 = np.eye(128, dtype=np.uint8)

    in_maps = []
    for b in range(B):
        in_maps.append({
            "xc": np.ascontiguousarray(xc_all[b]),
            "slabs": np.ascontiguousarray(slabs[b]),
            "wc": wc, "bc": bc, "wt": wt, "be": be, "idn": idn, "c0": c0,
        })
    return in_maps


def kernel(x, w_comp, b_comp, w_enc, b_enc, power_p):
    x = np.asarray(x, dtype=np.float32)
    in_maps = _host_prep(np.asarray(x), np.asarray(w_comp), np.asarray(b_comp),
                         np.asarray(w_enc), np.asarray(b_enc),
                         np.asarray(power_p))
    nc = _get_nc()
    res = run_bass_kernel_spmd(nc, in_maps, list(range(NCORES)))
    global LAST_RESULTS
    LAST_RESULTS = res
    outs = np.stack([np.asarray(res.results[i]["out"]) for i in range(NCORES)])
    # [B, 32, 128, 256] -> [B, C, 64, 64]; h' = half*32 + k, p = half*64 + w'
    out = (outs.reshape(B, NB, 2, 64, C)
               .transpose(0, 4, 2, 1, 3)
               .reshape(B, C, HP, WP))
    return np.ascontiguousarray(out.astype(np.float32))


# revision 21
# speedup vs baseline: 1.0707x; 1.0137x over previous
"""CARAFE-Downsample Trainium2 kernel (8 NeuronCores, data-parallel over batch).

Problem (hardcoded shapes): x [8, 256, 128, 128] f32; 1x1-conv compressor ->
cx [8, 64, 128, 128]; 3x3 stride-2 conv encoder -> mask [8, 25, 64, 64];
softmax(mask * exp(p)) over the 25 taps; 5x5 stride-2 weighted reassembly of x
-> out [8, 256, 64, 64].

Strategy (v1.1):
 - one sample per core (B == n_cores == 8).
 - Pixel-block layout: output block k (k in 0..31) holds the 128 output pixels
   {(h', w') : h' in {k, k+32}, w' in 0..63} on the 128 SBUF partitions
   (p = half*64 + w').  Every 5x5 tap is a single full-width PE matmul with a
   diagonal stationary matrix diag(e_t) against a host-pregathered slab.
 - diag matrices for all 25 taps of a block are written by ONE DVE
   copy_predicated into a [128, 25, 128] tile whose zero background is
   memset once per rotation buffer and never dirtied (the predicated write
   touches only the diagonal cells, which repeat every rotation).
 - softmax normalization is folded into the final psum->SBUF copy
   (ACT Copy with per-partition scale 1/sum(exp)); diag values are raw exps.
 - mask path (compressor + encoder) runs in bf16 on the TensorEngine;
   exp(power_p) folded into encoder weights on host.
 - DMA: xc loads batched in pairs, outputs batched 4 blocks per store,
   first slab tiles prefetched during phase A.
"""

import numpy as np
import ml_dtypes

import concourse.bass as bass
import concourse.bacc as bacc
import concourse.tile as tile
from concourse import mybir
from concourse.bass_utils import run_bass_kernel_spmd

# -- problem constants (hardcoded per spec) ---------------------------------
B, C, H, W = 8, 256, 128, 128
CC = 64           # compressed channels
KK = 5            # CARAFE window
HP = WP = 64      # output spatial
NB = 32           # pixel blocks per sample
NCORES = 8

X_DTYPE = "bf16"
MASK_DTYPE = "bf16"

_DTM = {"f32": mybir.dt.float32, "bf16": mybir.dt.bfloat16}
_NPM = {"f32": np.float32, "bf16": ml_dtypes.bfloat16}
DTX, DTK = _DTM[X_DTYPE], _DTM[MASK_DTYPE]
NPX, NPK = _NPM[X_DTYPE], _NPM[MASK_DTYPE]
F32 = mybir.dt.float32

# tap -> (slab index, block-row offset). slab sl = oh*5 + j holds x rows of
# parity oh, cols (j-2)+2*w'' (zero padded), block rows kk = -1..32.
def _tap_table():
    taps = []
    for i in range(KK):
        oh = (i - 2) % 2
        dh = (i - 2 - oh) // 2
        for j in range(KK):
            taps.append((i * 5 + j, oh * 5 + j, dh))
    return taps

_TAPS = _tap_table()

NSLAB_EARLY = 9   # slab tiles prefetched during phase A (== slab pool bufs)


def _build_nc():
    nc = bacc.Bacc(None, target_bir_lowering=False, debug=False)

    xc_d = nc.declare_dram_parameter("xc", [128, 2, H * W], DTK, isOutput=False)
    sl_d = nc.declare_dram_parameter("slabs", [34, 128, 10, C], DTX, isOutput=False)
    wc_d = nc.declare_dram_parameter("wc", [2, 128, CC], DTK, isOutput=False)
    bc_d = nc.declare_dram_parameter("bc", [CC, 1], F32, isOutput=False)
    wt_d = nc.declare_dram_parameter("wt", [CC, 9, 25], DTK, isOutput=False)
    be_d = nc.declare_dram_parameter("be", [25, 1], F32, isOutput=False)
    id_d = nc.declare_dram_parameter("idn", [25, 25], DTK, isOutput=False)
    c0_d = nc.declare_dram_parameter("c0", [128, 128], mybir.dt.uint8,
                                     isOutput=False)
    out_d = nc.declare_dram_parameter("out", [NB, 128, C], F32, isOutput=True)

    CXW = 130  # padded cx row length; cx_pad[c, r*130 + col], r/col offset by 1

    with tile.TileContext(nc) as tc:
        with (
            tc.tile_pool(name="consts", bufs=1) as consts,
            tc.tile_pool(name="xcin", bufs=6) as xcin,
            tc.tile_pool(name="cx", bufs=1) as cxpool,
            tc.tile_pool(name="psA", bufs=1, space="PSUM") as psA,
            tc.tile_pool(name="psM", bufs=2, space="PSUM") as psM,
            tc.tile_pool(name="psT", bufs=2, space="PSUM") as psT,
            tc.tile_pool(name="psO", bufs=3, space="PSUM") as psO,
            tc.tile_pool(name="soft", bufs=6) as soft,
            tc.tile_pool(name="slab", bufs=NSLAB_EARLY) as slabp,
            tc.tile_pool(name="diag", bufs=4) as diagp,
            tc.tile_pool(name="outp", bufs=2) as outp,
        ):
            # ---- constants / weights ----
            wc_sb = consts.tile([128, 2, CC], DTK)
            nc.sync.dma_start(out=wc_sb, in_=wc_d[:, :, :].rearrange("c p m -> p c m"))
            wt_sb = consts.tile([CC, 9, 25], DTK)
            nc.sync.dma_start(out=wt_sb, in_=wt_d[:, :, :])
            bc_sb = consts.tile([CC, 1], F32)
            nc.sync.dma_start(out=bc_sb, in_=bc_d[:, :])
            be_sb = consts.tile([25, 1], F32)
            nc.sync.dma_start(out=be_sb, in_=be_d[:, :])
            id_sb = consts.tile([25, 25], DTK)
            nc.sync.dma_start(out=id_sb, in_=id_d[:, :])
            c0_sb = consts.tile([128, 128], mybir.dt.uint8)
            nc.sync.dma_start(out=c0_sb, in_=c0_d[:, :])

            # ---- cx_pad (compressor output, 1-px zero ring, flat layout) ----
            cx_pad = cxpool.tile([CC, CXW * CXW], DTK)
            cp = cx_pad[:, :]
            zrow = consts.tile([CC, CXW], DTK)
            nc.vector.memset(zrow, 0.0)
            nc.scalar.copy(out=cp[:, 0:CXW], in_=zrow[:, :])
            nc.scalar.copy(
                out=bass.AP(tensor=cp.tensor, offset=cp.offset + CXW,
                            ap=[cp.ap[0], [CXW, 129], [1, 1]]),
                in_=zrow[:, 0:129],
            )

            tc.strict_bb_all_engine_barrier()

            # ---- phase A: compressor 1x1 conv (PE, bf16), xc in pairs ----
            slab_tiles = []

            def load_slab():
                kk = len(slab_tiles)
                st = slabp.tile([128, 10, C], DTX, tag="sl")
                nc.sync.dma_start(out=st, in_=sl_d[kk, :, :, :])
                slab_tiles.append(st)

            for jb in range(16):
                xt = xcin.tile([128, 2, 1024], DTK)
                nc.sync.dma_start(
                    out=xt, in_=xc_d[:, :, jb * 1024:(jb + 1) * 1024])
                for jj in range(2):
                    j = 2 * jb + jj
                    pm = psA.tile([CC, 512], F32)
                    nc.tensor.matmul(pm, lhsT=wc_sb[:, 0, :],
                                     rhs=xt[:, 0, jj * 512:(jj + 1) * 512],
                                     start=True, stop=False)
                    nc.tensor.matmul(pm, lhsT=wc_sb[:, 1, :],
                                     rhs=xt[:, 1, jj * 512:(jj + 1) * 512],
                                     start=False, stop=True)
                    dst = bass.AP(tensor=cp.tensor,
                                  offset=cp.offset + (4 * j + 1) * CXW + 1,
                                  ap=[cp.ap[0], [CXW, 4], [1, 128]])
                    if j % 2 == 0:
                        nc.scalar.activation(
                            out=dst,
                            in_=pm[:, :].rearrange("p (r n) -> p r n", n=128),
                            func=mybir.ActivationFunctionType.Identity,
                            bias=bc_sb[:, :])
                    else:
                        nc.vector.tensor_scalar(
                            out=dst,
                            in0=pm[:, :].rearrange("p (r n) -> p r n", n=128),
                            scalar1=bc_sb[:, :], scalar2=None,
                            op0=mybir.AluOpType.add)

            # ---- phase B: encoder 3x3/s2 conv -> m_all [25, 4096] (bf16) ----
            m_all = cxpool.tile([25, HP * WP], DTK)
            for j2 in (0, 4, 1, 5, 2, 6, 3, 7):
                pmM = psM.tile([25, 512], F32)
                ti = 0
                for di in range(3):
                    for dj in range(3):
                        rhs = bass.AP(
                            tensor=cp.tensor,
                            offset=cp.offset + (16 * j2 + di) * CXW + dj,
                            ap=[cp.ap[0], [2 * CXW, 8], [2, 64]],
                        )
                        nc.tensor.matmul(pmM, lhsT=wt_sb[:, ti, :], rhs=rhs,
                                         start=(ti == 0), stop=(ti == 8))
                        ti += 1
                nc.scalar.activation(out=m_all[:, j2 * 512:(j2 + 1) * 512],
                                     in_=pmM,
                                     func=mybir.ActivationFunctionType.Identity,
                                     bias=be_sb[:, :])

            for _ in range(34):
                load_slab()

            # ---- phase C: per block: transpose + exp + 1/sum ----
            e_blocks, r_blocks = [], []
            for k in range(NB):
                e_k = soft.tile([128, 25], F32, tag="e")
                for half in range(2):
                    hcol = (k + 32 * half) * 64
                    pmT = psT.tile([64, 25], DTK)
                    nc.tensor.transpose(pmT, m_all[:, hcol:hcol + 64], id_sb[:, :])
                    nc.scalar.activation(out=e_k[half * 64:(half + 1) * 64, :],
                                         in_=pmT,
                                         func=mybir.ActivationFunctionType.Exp)
                r_k = soft.tile([128, 1], F32, tag="r")
                nc.vector.reduce_sum(out=r_k, in_=e_k, axis=mybir.AxisListType.X)
                nc.vector.reciprocal(out=r_k, in_=r_k)
                e_blocks.append(e_k)
                r_blocks.append(r_k)

            # ---- phase D: diag-matmul reassembly ----
            tapmap = {t: (sl, dh) for (t, sl, dh) in _TAPS}
            c0_v = bass.AP(tensor=c0_sb.tensor, offset=c0_sb[:, :].offset,
                           ap=[c0_sb[:, :].ap[0], [0, 25], [1, 128]])

            # zero every physical diag buffer once; the predicated writes only
            # ever touch the diagonal cells, so the background stays zero
            for _ in range(4):
                Dz = diagp.tile([128, 25, 128], DTX, tag="diag")
                nc.vector.memset(Dz, 0.0)

            for k in range(NB):
                e_k, r_k = e_blocks[k], r_blocks[k]
                D_all = diagp.tile([128, 25, 128], DTX, tag="diag")
                ev = bass.AP(tensor=e_k.tensor, offset=e_k[:, :].offset,
                             ap=[e_k[:, :].ap[0], [1, 25], [0, 128]])
                nc.vector.copy_predicated(out=D_all, mask=c0_v, data=ev)
                po_t = psO.tile([128, C], F32)
                for t in range(25):
                    sl, dh = tapmap[t]
                    nc.tensor.matmul(po_t, lhsT=D_all[:, t, :],
                                     rhs=slab_tiles[k + dh + 1][:, sl, :],
                                     start=(t == 0), stop=(t == 24))
                if k % 4 == 0:
                    fin4 = outp.tile([128, 4, C], F32, tag="fin")
                nc.scalar.activation(out=fin4[:, k % 4, :], in_=po_t,
                                     func=mybir.ActivationFunctionType.Copy,
                                     scale=r_k[:, :])
                if k % 4 == 3:
                    nc.sync.dma_start(
                        out=out_d[k - 3:k + 1, :, :].rearrange("k p c -> p k c"),
                        in_=fin4)

    nc.compile()
    return nc


_NC_CACHE = None
LAST_RESULTS = None


def _get_nc():
    global _NC_CACHE
    if _NC_CACHE is None:
        _NC_CACHE = _build_nc()
    return _NC_CACHE


def _host_prep(x, w_comp, b_comp, w_enc, b_enc, power_p):
    """Build per-core input maps (numpy only)."""
    pe = float(np.exp(np.float64(power_p)))

    xc_all = np.ascontiguousarray(
        x.reshape(B, 2, 128, H * W).transpose(0, 2, 1, 3)).astype(NPK)

    # slabs [B, 34, 128, 10, C]
    xp = np.pad(x, ((0, 0), (0, 0), (2, 2), (2, 2)))  # [B, C, 132, 132]
    kk = np.arange(-1, 33)
    slabs = np.empty((B, 34, 128, 10, C), dtype=NPX)
    for oh in range(2):
        rows = (2 * kk[:, None] + 64 * np.arange(2)[None, :]) + oh + 2  # [34, 2]
        g0 = xp[:, :, rows, :]                     # [B, C, 34, 2, 132]
        for j in range(KK):
            g = g0[:, :, :, :, j:j + 128:2]        # [B, C, 34, 2, 64]
            slabs[:, :, :, oh * 5 + j, :] = (
                g.transpose(0, 2, 3, 4, 1).reshape(B, 34, 128, C))

    wc = np.ascontiguousarray(
        w_comp[:, :, 0, 0].T.reshape(2, 128, CC)).astype(NPK)
    bc = b_comp.reshape(CC, 1).astype(np.float32)
    wt = np.empty((CC, 9, 25), dtype=NPK)
    for di in range(3):
        for dj in range(3):
            wt[:, 3 * di + dj, :] = (pe * w_enc[:, :, di, dj]).T.astype(NPK)
    be = (pe * b_enc).reshape(25, 1).astype(np.float32)
    idn = np.eye(25, dtype=NPK)
    c0 = np.eye(128, dtype=np.uint8)

    in_maps = []
    for b in range(B):
        in_maps.append({
            "xc": np.ascontiguousarray(xc_all[b]),
            "slabs": np.ascontiguousarray(slabs[b]),
            "wc": wc, "bc": bc, "wt": wt, "be": be, "idn": idn, "c0": c0,
        })
    return in_maps


def kernel(x, w_comp, b_comp, w_enc, b_enc, power_p):
    x = np.asarray(x, dtype=np.float32)
    in_maps = _host_prep(np.asarray(x), np.asarray(w_comp), np.asarray(b_comp),
                         np.asarray(w_enc), np.asarray(b_enc),
                         np.asarray(power_p))
    nc = _get_nc()
    res = run_bass_kernel_spmd(nc, in_maps, list(range(NCORES)))
    global LAST_RESULTS
    LAST_RESULTS = res
    outs = np.stack([np.asarray(res.results[i]["out"]) for i in range(NCORES)])
    # [B, 32, 128, 256] -> [B, C, 64, 64]; h' = half*32 + k, p = half*64 + w'
    out = (outs.reshape(B, NB, 2, 64, C)
               .transpose(0, 4, 2, 1, 3)
               .reshape(B, C, HP, WP))
    return np.ascontiguousarray(out.astype(np.float32))


# revision 22
# speedup vs baseline: 1.1519x; 1.0759x over previous
"""CARAFE-Downsample Trainium2 kernel (8 NeuronCores, data-parallel over batch).

Problem (hardcoded shapes): x [8, 256, 128, 128] f32; 1x1-conv compressor ->
cx [8, 64, 128, 128]; 3x3 stride-2 conv encoder -> mask [8, 25, 64, 64];
softmax(mask * exp(p)) over the 25 taps; 5x5 stride-2 weighted reassembly of x
-> out [8, 256, 64, 64].

Strategy (v1.1):
 - one sample per core (B == n_cores == 8).
 - Pixel-block layout: output block k (k in 0..31) holds the 128 output pixels
   {(h', w') : h' in {k, k+32}, w' in 0..63} on the 128 SBUF partitions
   (p = half*64 + w').  Every 5x5 tap is a single full-width PE matmul with a
   diagonal stationary matrix diag(e_t) against a host-pregathered slab.
 - diag matrices for all 25 taps of a block are written by ONE DVE
   copy_predicated into a [128, 25, 128] tile whose zero background is
   memset once per rotation buffer and never dirtied (the predicated write
   touches only the diagonal cells, which repeat every rotation).
 - softmax normalization is folded into the final psum->SBUF copy
   (ACT Copy with per-partition scale 1/sum(exp)); diag values are raw exps.
 - mask path (compressor + encoder) runs in bf16 on the TensorEngine;
   exp(power_p) folded into encoder weights on host.
 - DMA: xc loads batched in pairs, outputs batched 4 blocks per store,
   first slab tiles prefetched during phase A.
"""

import numpy as np
import ml_dtypes

import concourse.bass as bass
import concourse.bacc as bacc
import concourse.tile as tile
from concourse import mybir
from concourse.bass_utils import run_bass_kernel_spmd

# -- problem constants (hardcoded per spec) ---------------------------------
B, C, H, W = 8, 256, 128, 128
CC = 64           # compressed channels
KK = 5            # CARAFE window
HP = WP = 64      # output spatial
NB = 32           # pixel blocks per sample
NCORES = 8

X_DTYPE = "bf16"
MASK_DTYPE = "bf16"

_DTM = {"f32": mybir.dt.float32, "bf16": mybir.dt.bfloat16}
_NPM = {"f32": np.float32, "bf16": ml_dtypes.bfloat16}
DTX, DTK = _DTM[X_DTYPE], _DTM[MASK_DTYPE]
NPX, NPK = _NPM[X_DTYPE], _NPM[MASK_DTYPE]
F32 = mybir.dt.float32

# tap -> (slab index, block-row offset). slab sl = oh*5 + j holds x rows of
# parity oh, cols (j-2)+2*w'' (zero padded), block rows kk = -1..32.
def _tap_table():
    taps = []
    for i in range(KK):
        oh = (i - 2) % 2
        dh = (i - 2 - oh) // 2
        for j in range(KK):
            taps.append((i * 5 + j, oh * 5 + j, dh))
    return taps

_TAPS = _tap_table()

NSLAB_EARLY = 9   # slab tiles prefetched during phase A (== slab pool bufs)


def _build_nc():
    nc = bacc.Bacc(None, target_bir_lowering=False, debug=False)

    xc_d = nc.declare_dram_parameter("xc", [128, 2, H * W], DTK, isOutput=False)
    sl_d = nc.declare_dram_parameter("slabs", [34, 128, 10, C], DTX, isOutput=False)
    wc_d = nc.declare_dram_parameter("wc", [2, 128, CC], DTK, isOutput=False)
    bc_d = nc.declare_dram_parameter("bc", [CC, 1], F32, isOutput=False)
    wt_d = nc.declare_dram_parameter("wt", [CC, 9, 25], DTK, isOutput=False)
    be_d = nc.declare_dram_parameter("be", [25, 1], F32, isOutput=False)
    id_d = nc.declare_dram_parameter("idn", [25, 25], DTK, isOutput=False)
    c0_d = nc.declare_dram_parameter("c0", [128, 128], mybir.dt.uint8,
                                     isOutput=False)
    out_d = nc.declare_dram_parameter("out", [NB, 128, C], F32, isOutput=True)

    CXW = 130  # padded cx row length; cx_pad[c, r*130 + col], r/col offset by 1

    with tile.TileContext(nc) as tc:
        with (
            tc.tile_pool(name="consts", bufs=1) as consts,
            tc.tile_pool(name="xcin", bufs=6) as xcin,
            tc.tile_pool(name="cx", bufs=1) as cxpool,
            tc.tile_pool(name="psA", bufs=2, space="PSUM") as psA,
            tc.tile_pool(name="psM", bufs=2, space="PSUM") as psM,
            tc.tile_pool(name="psT", bufs=2, space="PSUM") as psT,
            tc.tile_pool(name="psO", bufs=2, space="PSUM") as psO,
            tc.tile_pool(name="soft", bufs=6) as soft,
            tc.tile_pool(name="slab", bufs=NSLAB_EARLY) as slabp,
            tc.tile_pool(name="diag", bufs=4) as diagp,
            tc.tile_pool(name="outp", bufs=2) as outp,
        ):
            # ---- constants / weights ----
            wc_sb = consts.tile([128, 2, CC], DTK)
            nc.sync.dma_start(out=wc_sb, in_=wc_d[:, :, :].rearrange("c p m -> p c m"))
            wt_sb = consts.tile([CC, 9, 25], DTK)
            nc.sync.dma_start(out=wt_sb, in_=wt_d[:, :, :])
            bc_sb = consts.tile([CC, 1], F32)
            nc.sync.dma_start(out=bc_sb, in_=bc_d[:, :])
            be_sb = consts.tile([25, 1], F32)
            nc.sync.dma_start(out=be_sb, in_=be_d[:, :])
            id_sb = consts.tile([25, 25], DTK)
            nc.sync.dma_start(out=id_sb, in_=id_d[:, :])
            c0_sb = consts.tile([128, 128], mybir.dt.uint8)
            nc.sync.dma_start(out=c0_sb, in_=c0_d[:, :])

            # ---- cx_pad (compressor output, 1-px zero ring, flat layout) ----
            cx_pad = cxpool.tile([CC, CXW * CXW], DTK)
            cp = cx_pad[:, :]
            zrow = consts.tile([CC, CXW], DTK)
            nc.vector.memset(zrow, 0.0)
            nc.scalar.copy(out=cp[:, 0:CXW], in_=zrow[:, :])
            nc.scalar.copy(
                out=bass.AP(tensor=cp.tensor, offset=cp.offset + CXW,
                            ap=[cp.ap[0], [CXW, 129], [1, 1]]),
                in_=zrow[:, 0:129],
            )

            tc.strict_bb_all_engine_barrier()

            # ---- phase A: compressor 1x1 conv (PE, bf16), xc in pairs ----
            slab_tiles = []

            def load_slab():
                kk = len(slab_tiles)
                st = slabp.tile([128, 10, C], DTX, tag="sl")
                nc.sync.dma_start(out=st, in_=sl_d[kk, :, :, :])
                slab_tiles.append(st)

            for jb in range(16):
                xt = xcin.tile([128, 2, 1024], DTK)
                nc.sync.dma_start(
                    out=xt, in_=xc_d[:, :, jb * 1024:(jb + 1) * 1024])
                for jj in range(2):
                    j = 2 * jb + jj
                    pm = psA.tile([CC, 512], F32)
                    nc.tensor.matmul(pm, lhsT=wc_sb[:, 0, :],
                                     rhs=xt[:, 0, jj * 512:(jj + 1) * 512],
                                     start=True, stop=False)
                    nc.tensor.matmul(pm, lhsT=wc_sb[:, 1, :],
                                     rhs=xt[:, 1, jj * 512:(jj + 1) * 512],
                                     start=False, stop=True)
                    dst = bass.AP(tensor=cp.tensor,
                                  offset=cp.offset + (4 * j + 1) * CXW + 1,
                                  ap=[cp.ap[0], [CXW, 4], [1, 128]])
                    if j % 2 == 0:
                        nc.scalar.activation(
                            out=dst,
                            in_=pm[:, :].rearrange("p (r n) -> p r n", n=128),
                            func=mybir.ActivationFunctionType.Identity,
                            bias=bc_sb[:, :])
                    else:
                        nc.vector.tensor_scalar(
                            out=dst,
                            in0=pm[:, :].rearrange("p (r n) -> p r n", n=128),
                            scalar1=bc_sb[:, :], scalar2=None,
                            op0=mybir.AluOpType.add)

            # ---- phase B: encoder 3x3/s2 conv -> m_all [25, 4096] (bf16) ----
            m_all = cxpool.tile([25, HP * WP], DTK)
            for j2 in (0, 4, 1, 5, 2, 6, 3, 7):
                pmM = psM.tile([25, 512], F32)
                ti = 0
                for di in range(3):
                    for dj in range(3):
                        rhs = bass.AP(
                            tensor=cp.tensor,
                            offset=cp.offset + (16 * j2 + di) * CXW + dj,
                            ap=[cp.ap[0], [2 * CXW, 8], [2, 64]],
                        )
                        nc.tensor.matmul(pmM, lhsT=wt_sb[:, ti, :], rhs=rhs,
                                         start=(ti == 0), stop=(ti == 8))
                        ti += 1
                nc.scalar.activation(out=m_all[:, j2 * 512:(j2 + 1) * 512],
                                     in_=pmM,
                                     func=mybir.ActivationFunctionType.Identity,
                                     bias=be_sb[:, :])

            for _ in range(34):
                load_slab()

            # ---- phase C: per block: transpose + exp + 1/sum ----
            e_blocks, r_blocks = [], []
            for k in range(NB):
                e_k = soft.tile([128, 25], F32, tag="e")
                for half in range(2):
                    hcol = (k + 32 * half) * 64
                    pmT = psT.tile([64, 25], DTK)
                    nc.tensor.transpose(pmT, m_all[:, hcol:hcol + 64], id_sb[:, :])
                    nc.scalar.activation(out=e_k[half * 64:(half + 1) * 64, :],
                                         in_=pmT,
                                         func=mybir.ActivationFunctionType.Exp)
                r_k = soft.tile([128, 1], F32, tag="r")
                nc.vector.reduce_sum(out=r_k, in_=e_k, axis=mybir.AxisListType.X)
                nc.vector.reciprocal(out=r_k, in_=r_k)
                e_blocks.append(e_k)
                r_blocks.append(r_k)

            # ---- phase D: diag-matmul reassembly ----
            tapmap = {t: (sl, dh) for (t, sl, dh) in _TAPS}
            c0_v = bass.AP(tensor=c0_sb.tensor, offset=c0_sb[:, :].offset,
                           ap=[c0_sb[:, :].ap[0], [0, 25], [1, 128]])

            # zero every physical diag buffer once; the predicated writes only
            # ever touch the diagonal cells, so the background stays zero
            for _ in range(4):
                Dz = diagp.tile([128, 25, 128], DTX, tag="diag")
                nc.vector.memset(Dz, 0.0)

            for k in range(NB):
                e_k, r_k = e_blocks[k], r_blocks[k]
                D_all = diagp.tile([128, 25, 128], DTX, tag="diag")
                ev = bass.AP(tensor=e_k.tensor, offset=e_k[:, :].offset,
                             ap=[e_k[:, :].ap[0], [1, 25], [0, 128]])
                nc.vector.copy_predicated(out=D_all, mask=c0_v, data=ev)
                po_t = psO.tile([128, C], F32)
                for t in range(25):
                    sl, dh = tapmap[t]
                    nc.tensor.matmul(po_t, lhsT=D_all[:, t, :],
                                     rhs=slab_tiles[k + dh + 1][:, sl, :],
                                     start=(t == 0), stop=(t == 24))
                if k % 4 == 0:
                    fin4 = outp.tile([128, 4, C], F32, tag="fin")
                nc.scalar.activation(out=fin4[:, k % 4, :], in_=po_t,
                                     func=mybir.ActivationFunctionType.Copy,
                                     scale=r_k[:, :])
                if k % 4 == 3:
                    nc.sync.dma_start(
                        out=out_d[k - 3:k + 1, :, :].rearrange("k p c -> p k c"),
                        in_=fin4)

    nc.compile()
    return nc


_NC_CACHE = None
LAST_RESULTS = None


def _get_nc():
    global _NC_CACHE
    if _NC_CACHE is None:
        _NC_CACHE = _build_nc()
    return _NC_CACHE


def _host_prep(x, w_comp, b_comp, w_enc, b_enc, power_p):
    """Build per-core input maps (numpy only)."""
    pe = float(np.exp(np.float64(power_p)))

    xc_all = np.ascontiguousarray(
        x.reshape(B, 2, 128, H * W).transpose(0, 2, 1, 3)).astype(NPK)

    # slabs [B, 34, 128, 10, C]
    xp = np.pad(x, ((0, 0), (0, 0), (2, 2), (2, 2)))  # [B, C, 132, 132]
    kk = np.arange(-1, 33)
    slabs = np.empty((B, 34, 128, 10, C), dtype=NPX)
    for oh in range(2):
        rows = (2 * kk[:, None] + 64 * np.arange(2)[None, :]) + oh + 2  # [34, 2]
        g0 = xp[:, :, rows, :]                     # [B, C, 34, 2, 132]
        for j in range(KK):
            g = g0[:, :, :, :, j:j + 128:2]        # [B, C, 34, 2, 64]
            slabs[:, :, :, oh * 5 + j, :] = (
                g.transpose(0, 2, 3, 4, 1).reshape(B, 34, 128, C))

    wc = np.ascontiguousarray(
        w_comp[:, :, 0, 0].T.reshape(2, 128, CC)).astype(NPK)
    bc = b_comp.reshape(CC, 1).astype(np.float32)
    wt = np.empty((CC, 9, 25), dtype=NPK)
    for di in range(3):
        for dj in range(3):
            wt[:, 3 * di + dj, :] = (pe * w_enc[:, :, di, dj]).T.astype(NPK)
    be = (pe * b_enc).reshape(25, 1).astype(np.float32)
    idn = np.eye(25, dtype=NPK)
    c0 = np.eye(128, dtype=np.uint8)

    in_maps = []
    for b in range(B):
        in_maps.append({
            "xc": np.ascontiguousarray(xc_all[b]),
            "slabs": np.ascontiguousarray(slabs[b]),
            "wc": wc, "bc": bc, "wt": wt, "be": be, "idn": idn, "c0": c0,
        })
    return in_maps


def kernel(x, w_comp, b_comp, w_enc, b_enc, power_p):
    x = np.asarray(x, dtype=np.float32)
    in_maps = _host_prep(np.asarray(x), np.asarray(w_comp), np.asarray(b_comp),
                         np.asarray(w_enc), np.asarray(b_enc),
                         np.asarray(power_p))
    nc = _get_nc()
    res = run_bass_kernel_spmd(nc, in_maps, list(range(NCORES)))
    global LAST_RESULTS
    LAST_RESULTS = res
    outs = np.stack([np.asarray(res.results[i]["out"]) for i in range(NCORES)])
    # [B, 32, 128, 256] -> [B, C, 64, 64]; h' = half*32 + k, p = half*64 + w'
    out = (outs.reshape(B, NB, 2, 64, C)
               .transpose(0, 4, 2, 1, 3)
               .reshape(B, C, HP, WP))
    return np.ascontiguousarray(out.astype(np.float32))
